# revision 1
# baseline (speedup 1.0000x reference)
"""Memristive fully-connected layer on 8 Trainium2 NeuronCores.

Math: the reference interleaves pos/neg conductance columns, matmuls, and
takes the differential pair. Both columns of a pair see the same affine map
g = k_cond * w + G_OFF and the same voltages v = K_V * [x, 1], so in the
readout y = (I_pos - I_neg) / (K_V * k_cond) both G_OFF and k_cond cancel
exactly:

    y = x @ (w_pos - w_neg) + (b_pos - b_neg)

Sharding: tensor-parallel over the 1024 output columns (128 per core).
Each core reads x^T (shared), its [1024, 128] slices of w_pos/w_neg
(host-packed into one [1025, 256] array whose last row is the bias pair, so
each K-chunk is a single contiguous 128KB DMA), subtracts pos-neg on DVE,
and accumulates 8 K-chunk matmuls plus one K=1 bias-broadcast matmul into a
[128, 128] PSUM tile.

This walrus build admits only ONE sync wait per instruction, which shapes
the whole structure:
  - every tile gets its own slot (no WAR waits from slot reuse);
  - total DMA count stays <= 8 so the 8 round-robin DMAHW lanes are never
    reused (a reused lane would add a second wait);
  - two dummy N=1 "gate" matmuls make PE observe the two x^T DMA lanes, so
    each real matmul carries only its DVE (weight-subtract) wait;
  - bias constants are DVE-produced so the bias matmul waits on DVE alone;
  - Tile's multi-wait final drain is pruned to the output DMA's semaphore
    (everything else happens-before it); the sem-clear ISA op moves into
    the preamble and the second EVSEM barrier is dropped. The first
    barrier (per-engine dge_drain + EVSEM) stays so every engine quiesces
    its DMA state before its stream ends.

DMAs are issued weights-first on both HWDGE rings (SP and ACT) so the
fixed DGE completion latency overlaps compute, and the last-needed bytes
arrive as early as possible. CoreSim models ~8.1us/core; traffic is
~1.6MB/core against a ~358 GB/s HBM limit.
"""

import numpy as np

import concourse.bass as bass
import concourse.mybir as mybir
import concourse.tile as tile
from concourse.bass_utils import run_bass_kernel_spmd

B, NIN, NOUT = 128, 1024, 1024
NCORES = 8
NS = NOUT // NCORES  # output columns per core
KC = NIN // 128      # contraction chunks of 128
FP32 = mybir.dt.float32

_PROGRAM = None


def _prune_drain_waits(nc):
    """This walrus accepts at most ONE sync wait per instruction (any
    struct), but Tile's final drain carries one wait per semaphore. In this
    kernel every semaphore's final tick happens-before the output DMA's
    completion (inputs -> compute -> out copy -> y DMA form one chain), so
    the drain only needs the y DMA's completion semaphore. Keep exactly
    that wait and drop the rest."""
    y_sems = set()
    for f in nc.m.functions:
        for blk in f.blocks:
            for inst in blk.instructions:
                if type(inst).__name__ != "InstDMACopy":
                    continue
                si = inst.sync_info
                y_sems = {u.id for u in (si.on_update if si else [])}
    for f in nc.m.functions:
        for blk in f.blocks:
            for inst in blk.instructions:
                if type(inst).__name__ != "InstDrain":
                    continue
                si = inst.sync_info
                waits = list(si.on_wait) if si and si.on_wait else []
                if len(waits) <= 1:
                    continue
                keep = [w for w in waits if w.id in y_sems]
                assert keep, f"drain lost its y wait: {[w.ant_name for w in waits]}"
                inst.sync_info = mybir.SyncInfo(
                    on_wait=keep, on_update=list(si.on_update) if si else []
                )
    # safety: nothing else may exceed one wait
    for f in nc.m.functions:
        for blk in f.blocks:
            for inst in blk.instructions:
                si = getattr(inst, "sync_info", None)
                nw = len(si.on_wait) if si and si.on_wait else 0
                assert nw <= 1, (
                    f"{inst.name} ({type(inst).__name__}) has {nw} waits"
                )
    return nc


def _strip_tail(nc):
    """Tile's kernel tail is [drain][all-engine barrier][sem clear][barrier]
    (~2us). The pruned drain already guarantees the output DMA landed, and
    the EVSEM barrier sems self-reset, so the only state the tail must
    restore is the Tile semaphore range — move that single sem-clear ISA op
    into the preamble (before the first barrier) and drop everything after
    the drain. Each execution then starts from zeroed semaphores."""
    func = nc.m.functions[0]
    eb = [b for b in func.blocks if b.name.endswith("_end")][-1]
    insts = list(eb.instructions)
    isa_idx = next(
        i for i, inst in enumerate(insts) if type(inst).__name__ == "InstISA"
    )
    isa = insts[isa_idx]
    # keep the pruned drain AND the first all-engine barrier (per-engine
    # dge_drain + EVSEM) so every engine quiesces its DMA state before its
    # stream ends; drop only the sem clear (moved to preamble) and the
    # second barrier
    eb.instructions = insts[:isa_idx]

    mb = func.blocks[0]
    mi = list(mb.instructions)
    fi = next(
        i for i, inst in enumerate(mi) if type(inst).__name__ == "InstDrain"
    )
    mb.instructions = mi[:fi] + [isa] + mi[fi:]
    return nc


def _build(split=True):
    nc = bass.Bass()
    xt = nc.declare_dram_parameter("xt", [NIN, B], FP32, isOutput=False)
    w2 = nc.declare_dram_parameter("w2", [NIN + 1, 2 * NS], FP32, isOutput=False)
    y = nc.declare_dram_parameter("y", [B, NS], FP32, isOutput=True)

    with tile.TileContext(nc) as tc:
        with (
            tc.tile_pool(name="xpool", bufs=1) as xpool,
            tc.tile_pool(name="wpool", bufs=1) as wpool,
            tc.tile_pool(name="wdpool", bufs=1) as wdpool,
            tc.tile_pool(name="misc", bufs=1) as misc,
            tc.tile_pool(name="opool", bufs=1) as opool,
            tc.tile_pool(name="psum", bufs=1, space="PSUM") as psum_pool,
        ):
            # DMA schedule across the two HWDGE rings (SP=sync, ACT=scalar).
            # Each DMA's completion lags its issue by the fixed DGE latency,
            # so what matters is queue position: the first-needed tensors
            # (w chunks 0-1 and x^T chunks 0-3) go first on each ring; b2
            # (bias row, consumed last) goes last.
            #   sync  : xt_a | w1 | w3 | y
            #   scalar: w0 | xt_b | w2 | b2
            xt_r = xt[:].rearrange("(c p) m -> c p m", p=128)
            w2r = w2[0:NIN, :].rearrange("(d c p) n -> d p c n", p=128, c=2)

            w_tiles = []
            w_tiles.append(wpool.tile([128, 4 * NS], FP32, name="w0t", tag="w0"))
            nc.scalar.dma_start(
                w_tiles[0][:].rearrange("p (c n) -> p c n", c=2), w2r[0]
            )
            xt_a = xpool.tile([128, (KC // 2) * B], FP32, tag="xt_a")
            nc.sync.dma_start(
                xt_a[:].rearrange("p (c m) -> p c m", c=KC // 2),
                xt_r[0 : KC // 2].rearrange("c p m -> p c m"),
            )
            w_tiles.append(wpool.tile([128, 4 * NS], FP32, name="w1t", tag="w1"))
            nc.sync.dma_start(
                w_tiles[1][:].rearrange("p (c n) -> p c n", c=2), w2r[1]
            )
            xt_b = xpool.tile([128, (KC // 2) * B], FP32, tag="xt_b")
            nc.scalar.dma_start(
                xt_b[:].rearrange("p (c m) -> p c m", c=KC // 2),
                xt_r[KC // 2 : KC].rearrange("c p m -> p c m"),
            )
            w_tiles.append(wpool.tile([128, 4 * NS], FP32, name="w2t", tag="w2"))
            nc.scalar.dma_start(
                w_tiles[2][:].rearrange("p (c n) -> p c n", c=2), w2r[2]
            )
            w_tiles.append(wpool.tile([128, 4 * NS], FP32, name="w3t", tag="w3"))
            nc.sync.dma_start(
                w_tiles[3][:].rearrange("p (c n) -> p c n", c=2), w2r[3]
            )
            b2_t = misc.tile([1, 2 * NS], FP32)
            nc.scalar.dma_start(b2_t[:], w2[NIN : NIN + 1, :])

            def xt_chunk(c):
                t = xt_a if c < KC // 2 else xt_b
                lo = (c % (KC // 2)) * B
                return t[:, lo : lo + B]

            # bias difference and an all-ones row, both DVE-produced so the
            # bias matmul depends on the DVE semaphore alone
            bd_t = misc.tile([1, NS], FP32)
            nc.vector.tensor_sub(bd_t[:], b2_t[:, 0:NS], b2_t[:, NS : 2 * NS])
            ones_t = misc.tile([1, B], FP32)
            nc.vector.tensor_scalar(
                ones_t[:],
                b2_t[:, 0:B],
                0.0,
                1.0,
                mybir.AluOpType.mult,
                mybir.AluOpType.add,
            )

            ps = psum_pool.tile([B, NS], FP32)

            def emit_chunk(g, start):
                d, cl = g // 2, g % 2
                base = cl * 2 * NS
                wd_t = wdpool.tile([128, NS], FP32, name=f"wd{g}t", tag=f"wd{g}")
                nc.vector.tensor_sub(
                    wd_t[:],
                    w_tiles[d][:, base : base + NS],
                    w_tiles[d][:, base + NS : base + 2 * NS],
                )
                nc.tensor.matmul(
                    ps[:], xt_chunk(g), wd_t[:], start=start, stop=False
                )

            # PE warm-up: the HAM clock-gate keeps PE at 1.2 GHz until it
            # has seen ~3.4us of sustained activity. PE is otherwise idle
            # while the inputs stream in, so burn that window on dummy
            # matmuls over a DVE-memset tile; the real matmuls then run at
            # 2.4 GHz. Filler 1 waits on the DVE memset (one wait); the
            # rest reuse that observed tick.
            flt_t = misc.tile([128, B], FP32, name="flt")
            nc.vector.memset(flt_t[:], 1.0)
            flt_ps = psum_pool.tile([B, B], FP32, name="fltps")
            for _ in range(5):
                nc.tensor.matmul(
                    flt_ps[:], flt_t[:], flt_t[:], start=True, stop=True
                )

            # gate A: waits on xt_a's DMA lane only; chunks 0-3 then wait on
            # DVE alone. Gate B sits between chunk 3 and chunk 4 so it
            # cannot block the early matmuls.
            gate_ps = psum_pool.tile([B, 1], FP32)
            nc.tensor.matmul(
                gate_ps[:], xt_a[:, 0:B], xt_a[:, 0:1], start=True, stop=True
            )
            for g in range(KC // 2):
                emit_chunk(g, start=(g == 0))
            gate_ps2 = psum_pool.tile([B, 1], FP32)
            nc.tensor.matmul(
                gate_ps2[:], xt_b[:, 0:B], xt_b[:, 0:1], start=True, stop=True
            )
            for g in range(KC // 2, KC):
                emit_chunk(g, start=False)
            nc.tensor.matmul(ps[:], ones_t[:], bd_t[:], start=False, stop=True)

            out_t = opool.tile([B, NS], FP32)
            nc.vector.tensor_copy(out_t[:], ps[:])
            nc.sync.dma_start(y[:], out_t[:])
    return _strip_tail(_prune_drain_waits(nc)) if split else nc


def _program():
    global _PROGRAM
    if _PROGRAM is None:
        _PROGRAM = _build()
    return _PROGRAM


def _in_maps(x, w_pos, w_neg, b_pos, b_neg):
    x = np.ascontiguousarray(np.asarray(x, dtype=np.float32))
    w_pos = np.asarray(w_pos, dtype=np.float32)
    w_neg = np.asarray(w_neg, dtype=np.float32)
    b_pos = np.asarray(b_pos, dtype=np.float32)
    b_neg = np.asarray(b_neg, dtype=np.float32)
    xt = np.ascontiguousarray(x.T)
    maps = []
    for j in range(NCORES):
        sl = slice(j * NS, (j + 1) * NS)
        w2 = np.empty((NIN + 1, 2 * NS), dtype=np.float32)
        w2[:NIN, :NS] = w_pos[:, sl]
        w2[:NIN, NS:] = w_neg[:, sl]
        w2[NIN, :NS] = b_pos[sl]
        w2[NIN, NS:] = b_neg[sl]
        maps.append({"xt": xt, "w2": w2})
    return maps


def kernel(x, w_pos, w_neg, b_pos, b_neg):
    maps = _in_maps(x, w_pos, w_neg, b_pos, b_neg)
    res = run_bass_kernel_spmd(_program(), maps, list(range(NCORES))).results
    return np.concatenate([res[j]["y"] for j in range(NCORES)], axis=1)



# revision 2
# speedup vs baseline: 1.2240x; 1.2240x over previous
"""Memristive fully-connected layer on 8 Trainium2 NeuronCores.

Math: the reference interleaves pos/neg conductance columns, matmuls, and
takes the differential pair. Both columns of a pair see the same affine map
g = k_cond * w + G_OFF and the same voltages v = K_V * [x, 1], so in the
readout y = (I_pos - I_neg) / (K_V * k_cond) both G_OFF and k_cond cancel
exactly:

    y = x @ w_pos - x @ w_neg + (b_pos - b_neg)

Sharding: tensor-parallel over the 1024 output columns (128 per core).

v2 layout (vs the 8081ns v1): inputs are uploaded in fp16 (halving DMA
bytes; the pos/neg differential is taken AFTER both matmuls, in f32 PSUM,
so the fp16 rounding of w_pos/w_neg never cancels catastrophically —
measured rel err ~4e-3 against the 2e-2 gate) across THREE DGE queues
(SP + Activation HWDGE, Pool SWDGE). Each K-chunk matmul consumes its
weight tile straight from DMA into a [B, 2*NS] differential PSUM
(pos currents in cols 0:NS, neg in NS:2*NS); fp16 matmul retires 1 row/
cycle so a 256-wide chunk costs ~107ns at full clock. One Pool-engine
subtract ps[:, :NS] - ps[:, NS:] replaces v1's eight DVE weight
subtracts + copy, then SP DMAs y out.

The walrus one-wait-per-instruction discipline is kept from v1:
  - every tile has its own slot; each DMA queue is used <= 2 deep;
  - gate matmuls make PE observe the xta/xtb DMA semaphores, so chunk
    matmuls carry only their weight tile's wait;
  - the all-ones filler tile doubles as the bias matmul's stationary
    operand (its DVE memset semaphore is already observed by filler 1);
  - the bias pair row rides in the xtb tile's tail columns (no separate
    500ns-floor DMA), covered by gate B's wait;
  - Tile's final drain is pruned to the y DMA's semaphore and the kernel
    tail is stripped as in v1 (sem clear moved to the preamble).

PE warm-up fillers keep the tensor engine continuously busy from ~600ns so
the cost model's p-state ramp reaches full clock at t>3000ns before most
real matmuls issue.
"""

import numpy as np

import concourse.bass as bass
import concourse.mybir as mybir
import concourse.tile as tile
from concourse.bass_utils import run_bass_kernel_spmd

B, NIN, NOUT = 128, 1024, 1024
NCORES = 8
NS = NOUT // NCORES  # output columns per core
KC = NIN // 128      # contraction chunks of 128
FP32 = mybir.dt.float32
FP16 = mybir.dt.float16

_PROGRAM = None


def _prune_drain_waits(nc):
    """This walrus accepts at most ONE sync wait per instruction (any
    struct), but Tile's final drain carries one wait per semaphore. In this
    kernel every semaphore's final tick happens-before the output DMA's
    completion (inputs -> compute -> sub -> y DMA form one chain), so the
    drain only needs the y DMA's completion semaphore. Keep exactly that
    wait and drop the rest."""
    y_sems = set()
    for f in nc.m.functions:
        for blk in f.blocks:
            for inst in blk.instructions:
                if type(inst).__name__ != "InstDMACopy":
                    continue
                si = inst.sync_info
                y_sems = {u.id for u in (si.on_update if si else [])}
    for f in nc.m.functions:
        for blk in f.blocks:
            for inst in blk.instructions:
                if type(inst).__name__ != "InstDrain":
                    continue
                si = inst.sync_info
                waits = list(si.on_wait) if si and si.on_wait else []
                if len(waits) <= 1:
                    continue
                keep = [w for w in waits if w.id in y_sems]
                assert keep, f"drain lost its y wait: {[w.ant_name for w in waits]}"
                inst.sync_info = mybir.SyncInfo(
                    on_wait=keep, on_update=list(si.on_update) if si else []
                )
    # safety: nothing else may exceed one wait
    for f in nc.m.functions:
        for blk in f.blocks:
            for inst in blk.instructions:
                si = getattr(inst, "sync_info", None)
                nw = len(si.on_wait) if si and si.on_wait else 0
                assert nw <= 1, (
                    f"{inst.name} ({type(inst).__name__}) has {nw} waits"
                )
    return nc


def _strip_tail(nc):
    """Tile's kernel tail is [drain][all-engine barrier][sem clear][barrier]
    (~2us). The pruned drain already guarantees the output DMA landed, and
    the EVSEM barrier sems self-reset, so the only state the tail must
    restore is the Tile semaphore range — move that single sem-clear ISA op
    into the preamble (before the first barrier) and drop everything after
    the drain. Each execution then starts from zeroed semaphores."""
    func = nc.m.functions[0]
    eb = [b for b in func.blocks if b.name.endswith("_end")][-1]
    insts = list(eb.instructions)
    isa_idx = next(
        i for i, inst in enumerate(insts) if type(inst).__name__ == "InstISA"
    )
    isa = insts[isa_idx]
    # keep the pruned drain AND the first all-engine barrier (per-engine
    # dge_drain + EVSEM) so every engine quiesces its DMA state before its
    # stream ends; drop only the sem clear (moved to preamble) and the
    # second barrier
    eb.instructions = insts[:isa_idx]

    mb = func.blocks[0]
    mi = list(mb.instructions)
    fi = next(
        i for i, inst in enumerate(mi) if type(inst).__name__ == "InstDrain"
    )
    mb.instructions = mi[:fi] + [isa] + mi[fi:]
    return nc


def _build(split=True):
    nc = bass.Bass()
    xta = nc.declare_dram_parameter("xta", [128, 4 * B], FP16, isOutput=False)
    xtb = nc.declare_dram_parameter("xtb", [128, 4 * B + 2 * NS], FP16, isOutput=False)
    wa = nc.declare_dram_parameter("wa", [128, 4 * NS], FP16, isOutput=False)
    wb = nc.declare_dram_parameter("wb", [128, 4 * NS], FP16, isOutput=False)
    wc = nc.declare_dram_parameter("wc", [128, 8 * NS], FP16, isOutput=False)
    y = nc.declare_dram_parameter("y", [B, NS], FP32, isOutput=True)

    with tile.TileContext(nc) as tc:
        with (
            tc.tile_pool(name="xpool", bufs=1) as xpool,
            tc.tile_pool(name="wpool", bufs=1) as wpool,
            tc.tile_pool(name="misc", bufs=1) as misc,
            tc.tile_pool(name="opool", bufs=1) as opool,
            tc.tile_pool(name="psum", bufs=1, space="PSUM") as psum_pool,
        ):
            # DMA schedule: first-needed tensors take each queue's first
            # slot (fixed DGE latency then overlaps across queues).
            #   SP (sync)  : xta | xtb(+bias row) | y
            #   Act (scalar): wa | wb
            #   Pool (gpsimd SWDGE): wc
            xta_t = xpool.tile([128, 4 * B], FP16, tag="xta")
            nc.sync.dma_start(xta_t[:], xta[:])
            wa_t = wpool.tile([128, 4 * NS], FP16, tag="wa")
            nc.scalar.dma_start(wa_t[:], wa[:])
            wc_t = wpool.tile([128, 8 * NS], FP16, tag="wc")
            nc.gpsimd.dma_start(wc_t[:], wc[:])
            xtb_t = xpool.tile([128, 4 * B + 2 * NS], FP16, tag="xtb")
            nc.sync.dma_start(xtb_t[:], xtb[:])
            wb_t = wpool.tile([128, 4 * NS], FP16, tag="wb")
            nc.scalar.dma_start(wb_t[:], wb[:])

            # all-ones fp16 tile: PE warm-up filler moving operand AND the
            # bias matmul's stationary ones row (one DVE semaphore for both)
            flt_t = misc.tile([128, 4 * B], FP16, name="flt")
            nc.vector.memset(flt_t[:], 1.0)

            ps = psum_pool.tile([B, 2 * NS], FP32)

            # PE warm-up: keep PE continuously busy from ~600ns so the
            # p-state ramp reaches 2.4 GHz by t>3000 for the real matmuls.
            flt_ps = psum_pool.tile([B, 4 * B], FP32, name="fltps")
            for _ in range(4):
                nc.tensor.matmul(
                    flt_ps[:], flt_t[:, 0:B], flt_t[:], start=True, stop=True
                )

            def xt_chunk(c):
                t = xta_t if c < 4 else xtb_t
                lo = (c % 4) * B
                return t[:, lo : lo + B]

            # gate A: PE observes xta's DMA lane; chunks 0-3 then carry only
            # their weight tile's wait
            gate_ps = psum_pool.tile([B, 1], FP32)
            nc.tensor.matmul(
                gate_ps[:], xta_t[:, 0:B], xta_t[:, 0:1], start=True, stop=True
            )
            w_src = {0: wa_t, 1: wa_t, 2: wb_t, 3: wb_t,
                     4: wc_t, 5: wc_t, 6: wc_t, 7: wc_t}
            w_off = {0: 0, 1: 2 * NS, 2: 0, 3: 2 * NS,
                     4: 0, 5: 2 * NS, 6: 4 * NS, 7: 6 * NS}
            for g in range(4):
                nc.tensor.matmul(
                    ps[:],
                    xt_chunk(g),
                    w_src[g][:, w_off[g] : w_off[g] + 2 * NS],
                    start=(g == 0),
                    stop=False,
                )
            # gate B: PE observes xtb's DMA lane (covers chunks 4-7's
            # stationary operands AND the bias row in its tail columns)
            gate_ps2 = psum_pool.tile([B, 1], FP32)
            nc.tensor.matmul(
                gate_ps2[:], xtb_t[:, 0:B], xtb_t[:, 0:1], start=True, stop=True
            )
            for g in range(4, KC):
                nc.tensor.matmul(
                    ps[:],
                    xt_chunk(g),
                    w_src[g][:, w_off[g] : w_off[g] + 2 * NS],
                    start=False,
                    stop=False,
                )
            # bias pair row: ones[1,B] x bias[1,2*NS]; both operands'
            # semaphores already observed (DVE memset via filler 1, xtb via
            # gate B) so this carries no new wait
            nc.tensor.matmul(
                ps[:],
                flt_t[0:1, 0:B],
                xtb_t[0:1, 4 * B : 4 * B + 2 * NS],
                start=False,
                stop=True,
            )

            # differential pair readout on Pool (no PSUM access bubble
            # there), then y out on SP's idle queue
            out_t = opool.tile([B, NS], FP32)
            nc.gpsimd.tensor_sub(out_t[:], ps[:, 0:NS], ps[:, NS : 2 * NS])
            nc.sync.dma_start(y[:], out_t[:])
    return _strip_tail(_prune_drain_waits(nc)) if split else nc


def _program():
    global _PROGRAM
    if _PROGRAM is None:
        _PROGRAM = _build()
    return _PROGRAM


def _in_maps(x, w_pos, w_neg, b_pos, b_neg):
    x = np.asarray(x, dtype=np.float32)
    w_pos = np.asarray(w_pos, dtype=np.float32)
    w_neg = np.asarray(w_neg, dtype=np.float32)
    b_pos = np.asarray(b_pos, dtype=np.float32)
    b_neg = np.asarray(b_neg, dtype=np.float32)
    # x^T in K-chunk-major tile layout: chunk c cols hold x[:, c*128+p]^T
    xt = np.ascontiguousarray(x.T.astype(np.float16))  # [NIN, B]
    xt_r = xt.reshape(KC, 128, B)
    xta = np.ascontiguousarray(
        np.concatenate([xt_r[c] for c in range(4)], axis=1)
    )
    wp16 = w_pos.astype(np.float16)
    wn16 = w_neg.astype(np.float16)
    maps = []
    for j in range(NCORES):
        sl = slice(j * NS, (j + 1) * NS)
        xtb = np.zeros((128, 4 * B + 2 * NS), dtype=np.float16)
        xtb[:, : 4 * B] = np.concatenate(
            [xt_r[c] for c in range(4, KC)], axis=1
        )
        xtb[0, 4 * B : 4 * B + NS] = b_pos[sl].astype(np.float16)
        xtb[0, 4 * B + NS : 4 * B + 2 * NS] = b_neg[sl].astype(np.float16)

        def wtile(chunks):
            out = np.empty((128, len(chunks) * 2 * NS), dtype=np.float16)
            for i, c in enumerate(chunks):
                rows = slice(c * 128, (c + 1) * 128)
                out[:, i * 2 * NS : i * 2 * NS + NS] = wp16[rows, sl]
                out[:, i * 2 * NS + NS : (i + 1) * 2 * NS] = wn16[rows, sl]
            return out

        maps.append(
            {
                "xta": xta,
                "xtb": xtb,
                "wa": wtile([0, 1]),
                "wb": wtile([2, 3]),
                "wc": wtile([4, 5, 6, 7]),
            }
        )
    return maps


def kernel(x, w_pos, w_neg, b_pos, b_neg):
    maps = _in_maps(x, w_pos, w_neg, b_pos, b_neg)
    res = run_bass_kernel_spmd(_program(), maps, list(range(NCORES))).results
    return np.concatenate([res[j]["y"] for j in range(NCORES)], axis=1)


# revision 5
# speedup vs baseline: 1.2774x; 1.0436x over previous
"""Memristive fully-connected layer on 8 Trainium2 NeuronCores.

Math: the reference interleaves pos/neg conductance columns, matmuls, and
takes the differential pair. Both columns of a pair see the same affine map
g = k_cond * w + G_OFF and the same voltages v = K_V * [x, 1], so in the
readout y = (I_pos - I_neg) / (K_V * k_cond) both G_OFF and k_cond cancel
exactly:

    y = x @ w_pos - x @ w_neg + (b_pos - b_neg)

Sharding: tensor-parallel over the 1024 output columns (128 per core).

v2 layout (vs the 8081ns v1): inputs are uploaded in fp16 (halving DMA
bytes; the pos/neg differential is taken AFTER both matmuls, in f32 PSUM,
so the fp16 rounding of w_pos/w_neg never cancels catastrophically —
measured rel err ~4e-3 against the 2e-2 gate) across THREE DGE queues
(SP + Activation HWDGE, Pool SWDGE). Each K-chunk matmul consumes its
weight tile straight from DMA into a [B, 2*NS] differential PSUM
(pos currents in cols 0:NS, neg in NS:2*NS); fp16 matmul retires 1 row/
cycle so a 256-wide chunk costs ~107ns at full clock. One Pool-engine
subtract ps[:, :NS] - ps[:, NS:] replaces v1's eight DVE weight
subtracts + copy, then SP DMAs y out.

The walrus one-wait-per-instruction discipline is kept from v1:
  - every tile has its own slot; each DMA queue is used <= 2 deep;
  - gate matmuls make PE observe the xta/xtb DMA semaphores, so chunk
    matmuls carry only their weight tile's wait;
  - the all-ones filler tile doubles as the bias matmul's stationary
    operand (its DVE memset semaphore is already observed by filler 1);
  - the bias pair row rides in the xtb tile's tail columns (no separate
    500ns-floor DMA), covered by gate B's wait;
  - Tile's final drain is pruned to the y DMA's semaphore and the kernel
    tail is stripped as in v1 (sem clear moved to the preamble).

PE warm-up fillers keep the tensor engine continuously busy from ~600ns so
the cost model's p-state ramp reaches full clock at t>3000ns before most
real matmuls issue.
"""

import numpy as np

import concourse.bass as bass
import concourse.mybir as mybir
import concourse.tile as tile
from concourse.bass_utils import run_bass_kernel_spmd

B, NIN, NOUT = 128, 1024, 1024
NCORES = 8
NS = NOUT // NCORES  # output columns per core
KC = NIN // 128      # contraction chunks of 128
FP32 = mybir.dt.float32
FP16 = mybir.dt.float16

_PROGRAM = None


def _prune_drain_waits(nc):
    """This walrus accepts at most ONE sync wait per instruction (any
    struct), but Tile's final drain carries one wait per semaphore. In this
    kernel every semaphore's final tick happens-before the output DMA's
    completion (inputs -> compute -> sub -> y DMA form one chain), so the
    drain only needs the y DMA's completion semaphore. Keep exactly that
    wait and drop the rest."""
    y_sems = set()
    for f in nc.m.functions:
        for blk in f.blocks:
            for inst in blk.instructions:
                if type(inst).__name__ != "InstDMACopy":
                    continue
                si = inst.sync_info
                y_sems = {u.id for u in (si.on_update if si else [])}
    for f in nc.m.functions:
        for blk in f.blocks:
            for inst in blk.instructions:
                if type(inst).__name__ != "InstDrain":
                    continue
                si = inst.sync_info
                waits = list(si.on_wait) if si and si.on_wait else []
                if len(waits) <= 1:
                    continue
                keep = [w for w in waits if w.id in y_sems]
                assert keep, f"drain lost its y wait: {[w.ant_name for w in waits]}"
                inst.sync_info = mybir.SyncInfo(
                    on_wait=keep, on_update=list(si.on_update) if si else []
                )
    # safety: nothing else may exceed one wait
    for f in nc.m.functions:
        for blk in f.blocks:
            for inst in blk.instructions:
                si = getattr(inst, "sync_info", None)
                nw = len(si.on_wait) if si and si.on_wait else 0
                assert nw <= 1, (
                    f"{inst.name} ({type(inst).__name__}) has {nw} waits"
                )
    return nc


def _strip_tail(nc):
    """Tile's kernel tail is [drain][all-engine barrier][sem clear][barrier]
    (~2us). The pruned drain already guarantees the output DMA landed, and
    the EVSEM barrier sems self-reset, so the only state the tail must
    restore is the Tile semaphore range — move that single sem-clear ISA op
    into the preamble (before the first barrier) and drop everything after
    the drain. Each execution then starts from zeroed semaphores."""
    func = nc.m.functions[0]
    eb = [b for b in func.blocks if b.name.endswith("_end")][-1]
    insts = list(eb.instructions)
    isa_idx = next(
        i for i, inst in enumerate(insts) if type(inst).__name__ == "InstISA"
    )
    isa = insts[isa_idx]
    # keep the pruned drain and the per-engine dge_drains (each engine
    # quiesces its own DMA queues before its stream ends — on hardware the
    # drain op itself guarantees that engine's in-flight DMAs completed),
    # but drop the end-of-kernel EVSEM barrier: NRT only signals completion
    # once every engine stream has ended, so aligning the streams buys
    # nothing, and the next execution's preamble barrier re-syncs engines
    # after the semaphore clear. Barrier drains lose their release-sem
    # waits (the release EVSEMs are gone).
    kept = []
    for inst in insts[:isa_idx]:
        t = type(inst).__name__
        if t == "InstEventSemaphore":
            continue
        if t == "InstDrain":
            si = inst.sync_info
            waits = list(si.on_wait) if si and si.on_wait else []
            if any("barrier" in w.ant_name for w in waits):
                inst.sync_info = mybir.SyncInfo(on_wait=[], on_update=[])
        kept.append(inst)
    eb.instructions = kept

    mb = func.blocks[0]
    mi = list(mb.instructions)
    fi = next(
        i for i, inst in enumerate(mi) if type(inst).__name__ == "InstDrain"
    )
    mb.instructions = mi[:fi] + [isa] + mi[fi:]
    return nc


def _build(split=True):
    nc = bass.Bass()
    xta = nc.declare_dram_parameter("xta", [128, 4 * B], FP16, isOutput=False)
    xtb = nc.declare_dram_parameter("xtb", [128, 4 * B + 2 * NS], FP16, isOutput=False)
    wa = nc.declare_dram_parameter("wa", [128, 4 * NS], FP16, isOutput=False)
    wb = nc.declare_dram_parameter("wb", [128, 4 * NS], FP16, isOutput=False)
    wc = nc.declare_dram_parameter("wc", [128, 8 * NS], FP16, isOutput=False)
    y = nc.declare_dram_parameter("y", [B, NS], FP32, isOutput=True)

    with tile.TileContext(nc) as tc:
        with (
            tc.tile_pool(name="xpool", bufs=1) as xpool,
            tc.tile_pool(name="wpool", bufs=1) as wpool,
            tc.tile_pool(name="misc", bufs=1) as misc,
            tc.tile_pool(name="opool", bufs=1) as opool,
            tc.tile_pool(name="psum", bufs=1, space="PSUM") as psum_pool,
        ):
            # DMA schedule: first-needed tensors take each queue's first
            # slot (fixed DGE latency then overlaps across queues).
            #   SP (sync)  : xta | xtb(+bias row) | y
            #   Act (scalar): wa | wb
            #   Pool (gpsimd SWDGE): wc
            xta_t = xpool.tile([128, 4 * B], FP16, tag="xta")
            nc.sync.dma_start(xta_t[:], xta[:])
            wa_t = wpool.tile([128, 4 * NS], FP16, tag="wa")
            nc.scalar.dma_start(wa_t[:], wa[:])
            wc_t = wpool.tile([128, 8 * NS], FP16, tag="wc")
            nc.gpsimd.dma_start(wc_t[:], wc[:])
            xtb_t = xpool.tile([128, 4 * B + 2 * NS], FP16, tag="xtb")
            nc.sync.dma_start(xtb_t[:], xtb[:])
            wb_t = wpool.tile([128, 4 * NS], FP16, tag="wb")
            nc.scalar.dma_start(wb_t[:], wb[:])

            # all-ones fp16 tile: PE warm-up filler operand AND the bias
            # matmul's stationary ones row (one DVE semaphore for both);
            # kept narrow so the memset finishes fast and fillers start
            # by ~500ns
            flt_t = misc.tile([128, B], FP16, name="flt")
            nc.vector.memset(flt_t[:], 1.0)

            ps = psum_pool.tile([B, 2 * NS], FP32)

            # PE warm-up: keep PE continuously busy from ~500ns so the
            # p-state ramp reaches 2.4 GHz by t>3000 for the real matmuls;
            # sized to free PE just as xta's DMA lands (~2416ns)
            flt_ps = psum_pool.tile([B, B], FP32, name="fltps")
            for _ in range(18):
                nc.tensor.matmul(
                    flt_ps[:], flt_t[:], flt_t[:], start=True, stop=True
                )

            def xt_chunk(c):
                t = xta_t if c < 4 else xtb_t
                lo = (c % 4) * B
                return t[:, lo : lo + B]

            # gate A: PE observes xta's DMA lane; chunks 0-3 then carry only
            # their weight tile's wait
            gate_ps = psum_pool.tile([B, 1], FP32)
            nc.tensor.matmul(
                gate_ps[:], xta_t[:, 0:B], xta_t[:, 0:1], start=True, stop=True
            )
            w_src = {0: wa_t, 1: wa_t, 2: wb_t, 3: wb_t,
                     4: wc_t, 5: wc_t, 6: wc_t, 7: wc_t}
            w_off = {0: 0, 1: 2 * NS, 2: 0, 3: 2 * NS,
                     4: 0, 5: 2 * NS, 6: 4 * NS, 7: 6 * NS}
            for g in range(4):
                nc.tensor.matmul(
                    ps[:],
                    xt_chunk(g),
                    w_src[g][:, w_off[g] : w_off[g] + 2 * NS],
                    start=(g == 0),
                    stop=False,
                )
            # gate B: PE observes xtb's DMA lane (covers chunks 4-7's
            # stationary operands AND the bias row in its tail columns)
            gate_ps2 = psum_pool.tile([B, 1], FP32)
            nc.tensor.matmul(
                gate_ps2[:], xtb_t[:, 0:B], xtb_t[:, 0:1], start=True, stop=True
            )
            for g in range(4, KC):
                nc.tensor.matmul(
                    ps[:],
                    xt_chunk(g),
                    w_src[g][:, w_off[g] : w_off[g] + 2 * NS],
                    start=False,
                    stop=False,
                )
            # bias pair row: ones[1,B] x bias[1,2*NS]; both operands'
            # semaphores already observed (DVE memset via filler 1, xtb via
            # gate B) so this carries no new wait
            nc.tensor.matmul(
                ps[:],
                flt_t[0:1, :],
                xtb_t[0:1, 4 * B : 4 * B + 2 * NS],
                start=False,
                stop=True,
            )

            # differential pair readout on Pool (no PSUM access bubble
            # there), then y out on SP's idle queue
            out_t = opool.tile([B, NS], FP32)
            nc.gpsimd.tensor_sub(out_t[:], ps[:, 0:NS], ps[:, NS : 2 * NS])
            nc.sync.dma_start(y[:], out_t[:])
    return _strip_tail(_prune_drain_waits(nc)) if split else nc


def _program():
    global _PROGRAM
    if _PROGRAM is None:
        _PROGRAM = _build()
    return _PROGRAM


def _in_maps(x, w_pos, w_neg, b_pos, b_neg):
    x = np.asarray(x, dtype=np.float32)
    w_pos = np.asarray(w_pos, dtype=np.float32)
    w_neg = np.asarray(w_neg, dtype=np.float32)
    b_pos = np.asarray(b_pos, dtype=np.float32)
    b_neg = np.asarray(b_neg, dtype=np.float32)
    # x^T in K-chunk-major tile layout: chunk c cols hold x[:, c*128+p]^T
    xt = np.ascontiguousarray(x.T.astype(np.float16))  # [NIN, B]
    xt_r = xt.reshape(KC, 128, B)
    xta = np.ascontiguousarray(
        np.concatenate([xt_r[c] for c in range(4)], axis=1)
    )
    wp16 = w_pos.astype(np.float16)
    wn16 = w_neg.astype(np.float16)
    maps = []
    for j in range(NCORES):
        sl = slice(j * NS, (j + 1) * NS)
        xtb = np.zeros((128, 4 * B + 2 * NS), dtype=np.float16)
        xtb[:, : 4 * B] = np.concatenate(
            [xt_r[c] for c in range(4, KC)], axis=1
        )
        xtb[0, 4 * B : 4 * B + NS] = b_pos[sl].astype(np.float16)
        xtb[0, 4 * B + NS : 4 * B + 2 * NS] = b_neg[sl].astype(np.float16)

        def wtile(chunks):
            out = np.empty((128, len(chunks) * 2 * NS), dtype=np.float16)
            for i, c in enumerate(chunks):
                rows = slice(c * 128, (c + 1) * 128)
                out[:, i * 2 * NS : i * 2 * NS + NS] = wp16[rows, sl]
                out[:, i * 2 * NS + NS : (i + 1) * 2 * NS] = wn16[rows, sl]
            return out

        maps.append(
            {
                "xta": xta,
                "xtb": xtb,
                "wa": wtile([0, 1]),
                "wb": wtile([2, 3]),
                "wc": wtile([4, 5, 6, 7]),
            }
        )
    return maps


def kernel(x, w_pos, w_neg, b_pos, b_neg):
    maps = _in_maps(x, w_pos, w_neg, b_pos, b_neg)
    res = run_bass_kernel_spmd(_program(), maps, list(range(NCORES))).results
    return np.concatenate([res[j]["y"] for j in range(NCORES)], axis=1)


# revision 7
# speedup vs baseline: 1.2780x; 1.0005x over previous
"""Memristive fully-connected layer on 8 Trainium2 NeuronCores.

Math: the reference interleaves pos/neg conductance columns, matmuls, and
takes the differential pair. Both columns of a pair see the same affine map
g = k_cond * w + G_OFF and the same voltages v = K_V * [x, 1], so in the
readout y = (I_pos - I_neg) / (K_V * k_cond) both G_OFF and k_cond cancel
exactly:

    y = x @ w_pos - x @ w_neg + (b_pos - b_neg)

Sharding: tensor-parallel over the 1024 output columns (128 per core).

v2 layout (vs the 8081ns v1): inputs are uploaded in fp16 (halving DMA
bytes; the pos/neg differential is taken AFTER both matmuls, in f32 PSUM,
so the fp16 rounding of w_pos/w_neg never cancels catastrophically —
measured rel err ~4e-3 against the 2e-2 gate) across THREE DGE queues
(SP + Activation HWDGE, Pool SWDGE). Each K-chunk matmul consumes its
weight tile straight from DMA into a [B, 2*NS] differential PSUM
(pos currents in cols 0:NS, neg in NS:2*NS); fp16 matmul retires 1 row/
cycle so a 256-wide chunk costs ~107ns at full clock. One Pool-engine
subtract ps[:, :NS] - ps[:, NS:] replaces v1's eight DVE weight
subtracts + copy, then SP DMAs y out.

The walrus one-wait-per-instruction discipline is kept from v1:
  - every tile has its own slot; each DMA queue is used <= 2 deep;
  - gate matmuls make PE observe the xta/xtb DMA semaphores, so chunk
    matmuls carry only their weight tile's wait;
  - the all-ones filler tile doubles as the bias matmul's stationary
    operand (its DVE memset semaphore is already observed by filler 1);
  - the bias pair row rides in the xtb tile's tail columns (no separate
    500ns-floor DMA), covered by gate B's wait;
  - Tile's final drain is pruned to the y DMA's semaphore and the kernel
    tail is stripped as in v1 (sem clear moved to the preamble).

PE warm-up fillers keep the tensor engine continuously busy from ~600ns so
the cost model's p-state ramp reaches full clock at t>3000ns before most
real matmuls issue.
"""

import numpy as np

import concourse.bass as bass
import concourse.mybir as mybir
import concourse.tile as tile
from concourse.bass_utils import run_bass_kernel_spmd

B, NIN, NOUT = 128, 1024, 1024
NCORES = 8
NS = NOUT // NCORES  # output columns per core
KC = NIN // 128      # contraction chunks of 128
FP32 = mybir.dt.float32
FP16 = mybir.dt.float16

_PROGRAM = None


def _prune_drain_waits(nc):
    """This walrus accepts at most ONE sync wait per instruction (any
    struct), but Tile's final drain carries one wait per semaphore. In this
    kernel every semaphore's final tick happens-before the output DMA's
    completion (inputs -> compute -> sub -> y DMA form one chain), so the
    drain only needs the y DMA's completion semaphore. Keep exactly that
    wait and drop the rest."""
    y_sems = set()
    for f in nc.m.functions:
        for blk in f.blocks:
            for inst in blk.instructions:
                if type(inst).__name__ != "InstDMACopy":
                    continue
                si = inst.sync_info
                y_sems = {u.id for u in (si.on_update if si else [])}
    for f in nc.m.functions:
        for blk in f.blocks:
            for inst in blk.instructions:
                if type(inst).__name__ != "InstDrain":
                    continue
                si = inst.sync_info
                waits = list(si.on_wait) if si and si.on_wait else []
                if len(waits) <= 1:
                    continue
                keep = [w for w in waits if w.id in y_sems]
                assert keep, f"drain lost its y wait: {[w.ant_name for w in waits]}"
                inst.sync_info = mybir.SyncInfo(
                    on_wait=keep, on_update=list(si.on_update) if si else []
                )
    # safety: nothing else may exceed one wait
    for f in nc.m.functions:
        for blk in f.blocks:
            for inst in blk.instructions:
                si = getattr(inst, "sync_info", None)
                nw = len(si.on_wait) if si and si.on_wait else 0
                assert nw <= 1, (
                    f"{inst.name} ({type(inst).__name__}) has {nw} waits"
                )
    return nc


def _strip_tail(nc):
    """Tile's kernel tail is [drain][all-engine barrier][sem clear][barrier]
    (~2us). The pruned drain already guarantees the output DMA landed, and
    the EVSEM barrier sems self-reset, so the only state the tail must
    restore is the Tile semaphore range — move that single sem-clear ISA op
    into the preamble (before the first barrier) and drop everything after
    the drain. Each execution then starts from zeroed semaphores."""
    func = nc.m.functions[0]
    eb = [b for b in func.blocks if b.name.endswith("_end")][-1]
    insts = list(eb.instructions)
    isa_idx = next(
        i for i, inst in enumerate(insts) if type(inst).__name__ == "InstISA"
    )
    isa = insts[isa_idx]
    # keep the pruned drain and the per-engine dge_drains (each engine
    # quiesces its own DMA queues before its stream ends — on hardware the
    # drain op itself guarantees that engine's in-flight DMAs completed),
    # but drop the end-of-kernel EVSEM barrier: NRT only signals completion
    # once every engine stream has ended, so aligning the streams buys
    # nothing, and the next execution's preamble barrier re-syncs engines
    # after the semaphore clear. Barrier drains lose their release-sem
    # waits (the release EVSEMs are gone).
    kept = []
    for inst in insts[:isa_idx]:
        t = type(inst).__name__
        if t == "InstEventSemaphore":
            continue
        if t == "InstDrain":
            si = inst.sync_info
            waits = list(si.on_wait) if si and si.on_wait else []
            if any("barrier" in w.ant_name for w in waits):
                inst.sync_info = mybir.SyncInfo(on_wait=[], on_update=[])
        kept.append(inst)
    eb.instructions = kept

    mb = func.blocks[0]
    mi = list(mb.instructions)
    fi = next(
        i for i, inst in enumerate(mi) if type(inst).__name__ == "InstDrain"
    )
    mb.instructions = mi[:fi] + [isa] + mi[fi:]
    return nc


def _build(split=True):
    nc = bass.Bass()
    xta = nc.declare_dram_parameter("xta", [128, 4 * B], FP16, isOutput=False)
    xtb = nc.declare_dram_parameter("xtb", [128, 4 * B + 2 * NS], FP16, isOutput=False)
    wa = nc.declare_dram_parameter("wa", [128, 4 * NS], FP16, isOutput=False)
    wb = nc.declare_dram_parameter("wb", [128, 4 * NS], FP16, isOutput=False)
    wc = nc.declare_dram_parameter("wc", [128, 8 * NS], FP16, isOutput=False)
    y = nc.declare_dram_parameter("y", [B, NS], FP32, isOutput=True)

    with tile.TileContext(nc) as tc:
        with (
            tc.tile_pool(name="xpool", bufs=1) as xpool,
            tc.tile_pool(name="wpool", bufs=1) as wpool,
            tc.tile_pool(name="misc", bufs=1) as misc,
            tc.tile_pool(name="opool", bufs=1) as opool,
            tc.tile_pool(name="psum", bufs=1, space="PSUM") as psum_pool,
        ):
            # DMA schedule: first-needed tensors take each queue's first
            # slot (fixed DGE latency then overlaps across queues).
            #   SP (sync)  : xta | xtb(+bias row) | y
            #   Act (scalar): wa | wb
            #   Pool (gpsimd SWDGE): wc
            xta_t = xpool.tile([128, 4 * B], FP16, tag="xta")
            nc.sync.dma_start(xta_t[:], xta[:])
            wa_t = wpool.tile([128, 4 * NS], FP16, tag="wa")
            nc.scalar.dma_start(wa_t[:], wa[:])
            wc_t = wpool.tile([128, 8 * NS], FP16, tag="wc")
            nc.gpsimd.dma_start(wc_t[:], wc[:])
            xtb_t = xpool.tile([128, 4 * B + 2 * NS], FP16, tag="xtb")
            nc.sync.dma_start(xtb_t[:], xtb[:])
            wb_t = wpool.tile([128, 4 * NS], FP16, tag="wb")
            nc.scalar.dma_start(wb_t[:], wb[:])

            # all-ones fp16 row for the bias matmul's stationary operand
            # (the cost model's PE p-state ramp keys on absolute kernel
            # time, so no warm-up fillers are needed — verified empirically)
            flt_t = misc.tile([1, B], FP16, name="flt")
            nc.vector.memset(flt_t[:], 1.0)

            ps = psum_pool.tile([B, 2 * NS], FP32)

            def xt_chunk(c):
                t = xta_t if c < 4 else xtb_t
                lo = (c % 4) * B
                return t[:, lo : lo + B]

            # gate A: PE observes xta's DMA lane; chunks 0-3 then carry only
            # their weight tile's wait
            gate_ps = psum_pool.tile([B, 1], FP32)
            nc.tensor.matmul(
                gate_ps[:], xta_t[:, 0:B], xta_t[:, 0:1], start=True, stop=True
            )
            w_src = {0: wa_t, 1: wa_t, 2: wb_t, 3: wb_t,
                     4: wc_t, 5: wc_t, 6: wc_t, 7: wc_t}
            w_off = {0: 0, 1: 2 * NS, 2: 0, 3: 2 * NS,
                     4: 0, 5: 2 * NS, 6: 4 * NS, 7: 6 * NS}
            for g in range(4):
                nc.tensor.matmul(
                    ps[:],
                    xt_chunk(g),
                    w_src[g][:, w_off[g] : w_off[g] + 2 * NS],
                    start=(g == 0),
                    stop=False,
                )
            # gate B: PE observes xtb's DMA lane (covers chunks 4-7's
            # stationary operands AND the bias row in its tail columns)
            gate_ps2 = psum_pool.tile([B, 1], FP32)
            nc.tensor.matmul(
                gate_ps2[:], xtb_t[:, 0:B], xtb_t[:, 0:1], start=True, stop=True
            )
            for g in range(4, KC):
                nc.tensor.matmul(
                    ps[:],
                    xt_chunk(g),
                    w_src[g][:, w_off[g] : w_off[g] + 2 * NS],
                    start=False,
                    stop=False,
                )
            # bias pair row: ones[1,B] x bias[1,2*NS]; waits only the DVE
            # memset semaphore (xtb already observed via gate B)
            nc.tensor.matmul(
                ps[:],
                flt_t[:],
                xtb_t[0:1, 4 * B : 4 * B + 2 * NS],
                start=False,
                stop=True,
            )

            # differential pair readout on Pool (no PSUM access bubble
            # there), then y out on SP's idle queue
            out_t = opool.tile([B, NS], FP32)
            nc.gpsimd.tensor_sub(out_t[:], ps[:, 0:NS], ps[:, NS : 2 * NS])
            nc.sync.dma_start(y[:], out_t[:])
    return _strip_tail(_prune_drain_waits(nc)) if split else nc


def _program():
    global _PROGRAM
    if _PROGRAM is None:
        _PROGRAM = _build()
    return _PROGRAM


def _in_maps(x, w_pos, w_neg, b_pos, b_neg):
    x = np.asarray(x, dtype=np.float32)
    w_pos = np.asarray(w_pos, dtype=np.float32)
    w_neg = np.asarray(w_neg, dtype=np.float32)
    b_pos = np.asarray(b_pos, dtype=np.float32)
    b_neg = np.asarray(b_neg, dtype=np.float32)
    # x^T in K-chunk-major tile layout: chunk c cols hold x[:, c*128+p]^T
    xt = np.ascontiguousarray(x.T.astype(np.float16))  # [NIN, B]
    xt_r = xt.reshape(KC, 128, B)
    xta = np.ascontiguousarray(
        np.concatenate([xt_r[c] for c in range(4)], axis=1)
    )
    wp16 = w_pos.astype(np.float16)
    wn16 = w_neg.astype(np.float16)
    maps = []
    for j in range(NCORES):
        sl = slice(j * NS, (j + 1) * NS)
        xtb = np.zeros((128, 4 * B + 2 * NS), dtype=np.float16)
        xtb[:, : 4 * B] = np.concatenate(
            [xt_r[c] for c in range(4, KC)], axis=1
        )
        xtb[0, 4 * B : 4 * B + NS] = b_pos[sl].astype(np.float16)
        xtb[0, 4 * B + NS : 4 * B + 2 * NS] = b_neg[sl].astype(np.float16)

        def wtile(chunks):
            out = np.empty((128, len(chunks) * 2 * NS), dtype=np.float16)
            for i, c in enumerate(chunks):
                rows = slice(c * 128, (c + 1) * 128)
                out[:, i * 2 * NS : i * 2 * NS + NS] = wp16[rows, sl]
                out[:, i * 2 * NS + NS : (i + 1) * 2 * NS] = wn16[rows, sl]
            return out

        maps.append(
            {
                "xta": xta,
                "xtb": xtb,
                "wa": wtile([0, 1]),
                "wb": wtile([2, 3]),
                "wc": wtile([4, 5, 6, 7]),
            }
        )
    return maps


def kernel(x, w_pos, w_neg, b_pos, b_neg):
    maps = _in_maps(x, w_pos, w_neg, b_pos, b_neg)
    res = run_bass_kernel_spmd(_program(), maps, list(range(NCORES))).results
    return np.concatenate([res[j]["y"] for j in range(NCORES)], axis=1)
